# revision 11
# baseline (speedup 1.0000x reference)
"""TRN2 Bass kernel for DenseDilatedKnnGraph (B=4, C=64, N=4096, k=9, dilation=2).

Algorithm
---------
reference: xt (B,N,C); dist(i,j) = |xi|^2 - 2<xi,xj> + |xj|^2; nn_idx = top-18
of -dist per row (stable, lowest-index tie-break); output nn_idx[..., ::2] plus
a center-index row -> (2, B, N, 9) int32.

Per-row ordering of -dist is identical to the ordering of
    s_ij = 2<xi,xj> - |xj|^2
(the |xi|^2 term is constant per row).

Device (per core, SPMD over 8 cores; core = (batch, query-half)):
  - S computed via ONE fp16 K=67 matmul per 512-col chunk (stationary
    [qh(64); ones(3)], moving [ch(64); s1; s2; s3] where qh=fp16(2x_i),
    ch=fp16(x_j), s1..s3 = exact 3-level fp16 split of -|xj|^2).
    Values carry ~5e-3 absolute error from the fp16 rounding of q and c;
    selection errors this causes are detected host-side (margin flags)
    and repaired exactly.
  - DVE pass A: tensor_max pairs of PSUM columns (j, j+1024) per half
    -> SBUF fp32 [128,1024].  This is the cheapest possible crossing of
    the fp32 score stream (2 reads/lane/cycle).
  - GPSIMD pass B/C: continue the max tree 1024->512->256 per half ->
    one [128, 512] "slots" tile per query tile.  slot (h,j) = max of
    columns h*2048 + j + 256k, k=0..7.
  - DVE MAX8 (5 groups of ~102 slots) + MATCH/FIND_INDEX8 -> 40 slot ids
    per row (u16).  Only indices are DMA'd out.
  - Software-pipelined: the MAX8/FIND for tile t is emitted after pass A
    of tile t+1 so the DVE never stalls on the GPSIMD tree.

Host: each returned slot expands to its 8 member columns (320 candidates
per row); exact fp32 re-scoring + two-stage stable argsort reproduces the
jax top_k ordering (value desc, lowest index on ties).  Rows are repaired
by exact full recompute when (a) all 8 returned slots of some group score
>= v18 - eps (a 9th top-18 member may hide behind them), or (b) a FIND
duplicate collision at >= v18 - eps lost a slot.  Both checks follow from:
a true top-18 member can only be hidden by slots whose maxima are >= its
value (up to the device error bound eps).
"""

import numpy as np

import concourse.bacc as bacc
import concourse.mybir as mybir
import concourse.tile as tile
from concourse.bass_utils import run_bass_kernel_spmd

# Problem constants (hardcoded per harness contract).
B = 4
C = 64
N = 4096
K = 9
DILATION = 2
K_EFF = K * DILATION      # 18
P = 128                   # partitions / queries per tile
KM = C + 3                # matmul contraction: 64 q rows + 3 xsq rows
N_CORES = 8
QROWS = (B * N) // N_CORES          # 2048 query rows per core
N_TILES = QROWS // P                # 16 tiles per core

FMERGE = 16               # columns folded into one slot by the max tree
SLOTS = N // FMERGE       # 256 slots per row
HSLOT = SLOTS // 2        # 128 slots per half
# MAX8 group boundaries over slots.
GROUP_BOUNDS = (0, 64, 128, 192, 256)
NG = len(GROUP_BOUNDS) - 1
UW = NG * 8               # selected slots per row (40)
EPS = 0.06                # device value error bound for host flags
C0 = 96.0                 # score bias: centers top scores near 0 for fp16


def _build_program(n_tiles=N_TILES):
    nc = bacc.Bacc(
        "TRN2", target_bir_lowering=False, debug=False, enable_asserts=False
    )
    f32 = mybir.dt.float32
    f16 = mybir.dt.float16
    u16 = mybir.dt.uint16
    nq = n_tiles * P
    lhs = nc.dram_tensor("lhs", (KM, nq), f16, kind="ExternalInput")
    rhs = nc.dram_tensor("rhs", (KM, N), f16, kind="ExternalInput")
    l_out = nc.dram_tensor("l_out", (nq, UW), u16, kind="ExternalOutput")
    lhs_ap, rhs_ap, l_ap = lhs.ap(), rhs.ap(), l_out.ap()

    with tile.TileContext(nc) as tc:
        with (
            tc.tile_pool(name="const", bufs=1) as cpool,
            tc.tile_pool(name="psum", bufs=2, space="PSUM") as ppool,
            tc.tile_pool(name="sbf", bufs=3) as sbpool,
            tc.tile_pool(name="m1p", bufs=2) as m1pool,
            tc.tile_pool(name="m2p", bufs=2) as m2pool,
            tc.tile_pool(name="m3p", bufs=2) as m3pool,
            tc.tile_pool(name="slotp", bufs=3) as spool,
            tc.tile_pool(name="outp", bufs=3) as opool,
        ):
            # dependency-free warm-up matmuls that run during the input-DMA
            # prologue (nudges the PE toward its full-rate mode)
            prime = cpool.tile([KM, 512], f16)
            nc.gpsimd.memset(prime[:, :], 0.0)
            pps = ppool.tile([P, N // 2], f32, tag="ps")
            for _ in range(12):
                nc.tensor.matmul(pps[:, :512], prime[:, :128], prime[:, :],
                                 start=True, stop=True)

            # per-512-column-chunk rhs tiles: the first matmul only waits
            # for its own chunk, not the whole load
            r_sb = [
                cpool.tile([KM, 512], f16, name=f"r{j}", tag=f"r{j}")
                for j in range(8)
            ]
            l_sb = cpool.tile([KM, nq], f16)
            w0 = min(512, nq)
            nc.sync.dma_start(l_sb[:, 0:w0], lhs_ap[:, 0:w0])
            for j in range(8):
                nc.sync.dma_start(r_sb[j][:, :], rhs_ap[:, j * 512 : (j + 1) * 512])
            for j in range(512, nq, 512):
                w = min(512, nq - j)
                nc.sync.dma_start(l_sb[:, j : j + w], lhs_ap[:, j : j + w])

            def emit_maxfind(t, slot_t):
                u = opool.tile([P, UW], f16, tag="u")
                lo = opool.tile([P, UW], u16, tag="l")
                for g in range(NG):
                    nc.vector.max(
                        out=u[:, g * 8 : (g + 1) * 8],
                        in_=slot_t[:, GROUP_BOUNDS[g] : GROUP_BOUNDS[g + 1]],
                    )
                for g in range(NG):
                    nc.vector.max_index(
                        out=lo[:, g * 8 : (g + 1) * 8],
                        in_max=u[:, g * 8 : (g + 1) * 8],
                        in_values=slot_t[:, GROUP_BOUNDS[g] : GROUP_BOUNDS[g + 1]],
                    )
                rs = slice(t * P, (t + 1) * P)
                nc.sync.dma_start(l_ap[rs, :], lo[:])

            prev = None  # (t, slots tile) pending MAX8/FIND
            for t in range(n_tiles):
                qs = slice(t * P, (t + 1) * P)
                slot_t = spool.tile([P, SLOTS], f16, tag="slots")
                sbf = sbpool.tile([P, N], f16, tag="sbf")
                for h in range(2):
                    ps = ppool.tile([P, N // 2], f32, tag="ps")
                    for j in range(4):
                        cj = h * 4 + j
                        nc.tensor.matmul(
                            ps[:, j * 512 : (j + 1) * 512],
                            l_sb[:, qs], r_sb[cj][:, :],
                            start=True, stop=True,
                        )
                    # scalar engine stages the half into SBUF as fp16; the
                    # DVE max tree then runs in 2x (16-bit) mode
                    nc.scalar.copy(sbf[:, h * 2048 : (h + 1) * 2048], ps[:, :])
                # max tree, one batched instruction per level: view
                # [128, 2 halves, 2, W] and fold the pair dim
                m1 = m1pool.tile([P, 2048], f16, tag="m1")
                m2 = m2pool.tile([P, 1024], f16, tag="m2")
                m3 = m3pool.tile([P, 512], f16, tag="m3")
                for src, dst, w in (
                    (sbf, m1, 1024), (m1, m2, 512), (m2, m3, 256), (m3, slot_t, 128),
                ):
                    v = src[:, :].rearrange("p (h k w) -> p h k w", h=2, k=2)
                    nc.vector.tensor_max(
                        dst[:, :].rearrange("p (h w) -> p h w", h=2),
                        v[:, :, 0, :],
                        v[:, :, 1, :],
                    )
                if prev is not None:
                    emit_maxfind(*prev)
                prev = (t, slot_t)
            emit_maxfind(*prev)
    nc.compile()
    return nc


def _prep_core_inputs(X, core):
    """X: (B, N, C) fp32. Returns input map for one core."""
    b, h = divmod(core, N_CORES // B)
    Xb = X[b]
    xsq = np.sum(Xb * Xb, axis=1, dtype=np.float32)
    # 3-level fp16 split of (C0 - xsq) (exact to ~1e-6); the C0 bias
    # centers the top scores near 0 where the fp16 grid is finest
    t0 = C0 - xsq
    s1 = t0.astype(np.float16)
    r = t0 - s1.astype(np.float32)
    s2 = r.astype(np.float16)
    s3 = (r - s2.astype(np.float32)).astype(np.float16)
    rhs = np.empty((KM, N), np.float16)
    rhs[:C] = Xb.T.astype(np.float16)
    rhs[C] = s1
    rhs[C + 1] = s2
    rhs[C + 2] = s3
    lhs = np.empty((KM, QROWS), np.float16)
    lhs[:C] = (2.0 * Xb[h * QROWS : (h + 1) * QROWS]).T.astype(np.float16)
    lhs[C:] = 1.0
    return {"lhs": lhs, "rhs": rhs}


# base slot id of the group each of the UW output columns belongs to
_GROUP_BASE = np.asarray(GROUP_BOUNDS[:-1], dtype=np.int64)[np.arange(UW) // 8]


def _merge_core(L, Xb, xsq, h):
    """L: (QROWS, UW) u16 slot-local ids for one core. Returns
    (top18 (QROWS,18) int64, flagged row mask (QROWS,))."""
    R = L.shape[0]
    slots = L.astype(np.int64) + _GROUP_BASE[None, :]          # (R, UW)
    base_col = (slots >> 7) * 2048 + (slots & 127)             # (R, UW)
    cand = (base_col[:, :, None] + 128 * np.arange(FMERGE)[None, None, :]
            ).reshape(R, UW * FMERGE)                          # (R, 512)
    Q = 2.0 * Xb[h * QROWS : (h + 1) * QROWS]                  # (R, C)
    # exact scores: v[r,m] = <Q[r], X[cand]> - xsq[cand], chunked gather
    v = np.empty((R, UW * FMERGE), np.float32)
    CH = 512
    for r0 in range(0, R, CH):
        r1 = min(r0 + CH, R)
        Xg = Xb[cand[r0:r1]]                                   # (ch, 320, C)
        v[r0:r1] = np.matmul(Xg, Q[r0:r1, :, None])[..., 0]
    v -= xsq[cand]

    # stable jax-style ordering: by value desc, lowest column id on ties
    ord1 = np.argsort(cand, axis=1, kind="stable")
    cand1 = np.take_along_axis(cand, ord1, axis=1)
    v1 = np.take_along_axis(v, ord1, axis=1)
    ord2 = np.argsort(-v1, axis=1, kind="stable")
    top = np.take_along_axis(cand1, ord2, axis=1)[:, :K_EFF]
    v18 = np.take_along_axis(v1, ord2, axis=1)[:, K_EFF - 1]

    # flags
    slotmax = v.reshape(R, UW, FMERGE).max(axis=2)             # (R, UW)
    thr = (v18 - EPS)[:, None]
    cnt = (slotmax >= thr).reshape(R, NG, 8).sum(axis=2)       # (R, NG)
    flag_count = (cnt >= 8).any(axis=1)
    s_sorted = np.sort(slots.reshape(R, NG, 8), axis=2)
    sm_sorted = np.take_along_axis(
        slotmax.reshape(R, NG, 8), np.argsort(slots.reshape(R, NG, 8), axis=2), axis=2
    )
    dup = (np.diff(s_sorted, axis=2) == 0) & (sm_sorted[:, :, 1:] >= thr[:, :, None])
    flag_dup = dup.any(axis=(1, 2))
    return top, flag_count | flag_dup


_NC_CACHE = {}


def kernel(x: np.ndarray) -> np.ndarray:
    x = np.asarray(x)
    assert x.shape == (B, C, N, 1), x.shape
    X = np.ascontiguousarray(np.transpose(x[..., 0], (0, 2, 1)))  # (B, N, C)

    if N_TILES not in _NC_CACHE:
        _NC_CACHE[N_TILES] = _build_program(N_TILES)
    nc = _NC_CACHE[N_TILES]

    in_maps = [_prep_core_inputs(X, c) for c in range(N_CORES)]
    res = run_bass_kernel_spmd(nc, in_maps, core_ids=list(range(N_CORES)))

    xsqs = [np.sum(X[b] * X[b], axis=1, dtype=np.float32) for b in range(B)]
    nn_idx = np.empty((B, N, K_EFF), np.int64)
    bad_rows = [[] for _ in range(B)]
    for core in range(N_CORES):
        b, h = divmod(core, N_CORES // B)
        L = res.results[core]["l_out"]
        idx, bad = _merge_core(L, X[b], xsqs[b], h)
        nn_idx[b, h * QROWS : (h + 1) * QROWS] = idx
        if bad.any():
            bad_rows[b].extend((h * QROWS + np.nonzero(bad)[0]).tolist())

    # vectorized host repair of flagged rows (exact fp32 recompute)
    for b in range(B):
        if not bad_rows[b]:
            continue
        rows = np.asarray(sorted(bad_rows[b]))
        Xb = X[b]
        S = (2.0 * Xb[rows]) @ Xb.T
        S = (S - xsqs[b][None, :]).astype(np.float32)
        order = np.argsort(-S, axis=1, kind="stable")
        nn_idx[b, rows] = order[:, :K_EFF]

    nn_dil = nn_idx[:, :, ::DILATION]                       # (B, N, 9)
    center = np.broadcast_to(np.arange(N)[None, :, None], nn_dil.shape)
    out = np.stack((nn_dil, center), axis=0).astype(np.int32)
    return out


# revision 15
# speedup vs baseline: 1.1482x; 1.1482x over previous
"""TRN2 Bass kernel for DenseDilatedKnnGraph (B=4, C=64, N=4096, k=9, dilation=2).

Algorithm
---------
reference: xt (B,N,C); dist(i,j) = |xi|^2 - 2<xi,xj> + |xj|^2; nn_idx = top-18
of -dist per row (stable, lowest-index tie-break); output nn_idx[..., ::2] plus
a center-index row -> (2, B, N, 9) int32.

Per-row ordering of -dist is identical to the ordering of
    s_ij = 2<xi,xj> - |xj|^2
(the |xi|^2 term is constant per row).

Device (per core, SPMD over 8 cores; core = (batch, query-half)):
  - S computed via ONE fp16 K=67 matmul per 512-col chunk (stationary
    [qh(64); ones(3)], moving [ch(64); s1; s2; s3] where qh=fp16(2x_i),
    ch=fp16(x_j), s1..s3 = exact 3-level fp16 split of -|xj|^2).
    Values carry ~5e-3 absolute error from the fp16 rounding of q and c;
    selection errors this causes are detected host-side (margin flags)
    and repaired exactly.
  - DVE pass A: tensor_max pairs of PSUM columns (j, j+1024) per half
    -> SBUF fp32 [128,1024].  This is the cheapest possible crossing of
    the fp32 score stream (2 reads/lane/cycle).
  - GPSIMD pass B/C: continue the max tree 1024->512->256 per half ->
    one [128, 512] "slots" tile per query tile.  slot (h,j) = max of
    columns h*2048 + j + 256k, k=0..7.
  - DVE MAX8 (5 groups of ~102 slots) + MATCH/FIND_INDEX8 -> 40 slot ids
    per row (u16).  Only indices are DMA'd out.
  - Software-pipelined: the MAX8/FIND for tile t is emitted after pass A
    of tile t+1 so the DVE never stalls on the GPSIMD tree.

Host: each returned slot expands to its 8 member columns (320 candidates
per row); exact fp32 re-scoring + two-stage stable argsort reproduces the
jax top_k ordering (value desc, lowest index on ties).  Rows are repaired
by exact full recompute when (a) all 8 returned slots of some group score
>= v18 - eps (a 9th top-18 member may hide behind them), or (b) a FIND
duplicate collision at >= v18 - eps lost a slot.  Both checks follow from:
a true top-18 member can only be hidden by slots whose maxima are >= its
value (up to the device error bound eps).
"""

import numpy as np

import concourse.bacc as bacc
import concourse.mybir as mybir
import concourse.tile as tile
from concourse.bass_utils import run_bass_kernel_spmd

# Problem constants (hardcoded per harness contract).
B = 4
C = 64
N = 4096
K = 9
DILATION = 2
K_EFF = K * DILATION      # 18
P = 128                   # partitions / queries per tile
KM = C + 3                # matmul contraction: 64 q rows + 3 xsq rows
N_CORES = 8
QROWS = (B * N) // N_CORES          # 2048 query rows per core
N_TILES = QROWS // P                # 16 tiles per core

FMERGE = 16               # columns folded into one slot by the max tree
SLOTS = N // FMERGE       # 256 slots per row
HSLOT = SLOTS // 2        # 128 slots per half
# MAX8 group boundaries over slots.
GROUP_BOUNDS = (0, 64, 128, 192, 256)
NG = len(GROUP_BOUNDS) - 1
UW = NG * 8               # selected slots per row (40)
EPS = 0.06                # device value error bound for host flags
C0 = 96.0                 # score bias: centers top scores near 0 for fp16


def _build_program(n_tiles=N_TILES):
    nc = bacc.Bacc(
        "TRN2", target_bir_lowering=False, debug=False, enable_asserts=False
    )
    f32 = mybir.dt.float32
    f16 = mybir.dt.float16
    u16 = mybir.dt.uint16
    nq = n_tiles * P
    lhs = nc.dram_tensor("lhs", (KM, nq), f16, kind="ExternalInput")
    rhs = nc.dram_tensor("rhs", (KM, N), f16, kind="ExternalInput")
    l_out = nc.dram_tensor("l_out", (nq, UW), u16, kind="ExternalOutput")
    lhs_ap, rhs_ap, l_ap = lhs.ap(), rhs.ap(), l_out.ap()

    with tile.TileContext(nc) as tc:
        with (
            tc.tile_pool(name="const", bufs=1) as cpool,
            tc.tile_pool(name="psum", bufs=2, space="PSUM") as ppool,
            tc.tile_pool(name="sbf", bufs=3) as sbpool,
            tc.tile_pool(name="m1p", bufs=2) as m1pool,
            tc.tile_pool(name="m2p", bufs=2) as m2pool,
            tc.tile_pool(name="m3p", bufs=2) as m3pool,
            tc.tile_pool(name="slotp", bufs=3) as spool,
            tc.tile_pool(name="outp", bufs=3) as opool,
        ):
            # dependency-free warm-up matmuls that run during the input-DMA
            # prologue (nudges the PE toward its full-rate mode)
            prime = cpool.tile([KM, 512], f16)
            nc.gpsimd.memset(prime[:, :], 0.0)
            pps = ppool.tile([P, N // 2], f32, tag="ps")
            for _ in range(12):
                nc.tensor.matmul(pps[:, :512], prime[:, :128], prime[:, :],
                                 start=True, stop=True)

            # per-512-column-chunk rhs tiles: the first matmul only waits
            # for its own chunk, not the whole load
            r_sb = [
                cpool.tile([KM, 512], f16, name=f"r{j}", tag=f"r{j}")
                for j in range(8)
            ]
            l_sb = cpool.tile([KM, nq], f16)
            w0 = min(512, nq)
            nc.sync.dma_start(l_sb[:, 0:w0], lhs_ap[:, 0:w0])
            for j in range(8):
                nc.sync.dma_start(r_sb[j][:, :], rhs_ap[:, j * 512 : (j + 1) * 512])
            for j in range(512, nq, 512):
                w = min(512, nq - j)
                nc.sync.dma_start(l_sb[:, j : j + w], lhs_ap[:, j : j + w])

            def emit_maxfind(t, slot_t):
                u = opool.tile([P, UW], f16, tag="u")
                lo = opool.tile([P, UW], u16, tag="l")
                for g in range(NG):
                    nc.vector.max(
                        out=u[:, g * 8 : (g + 1) * 8],
                        in_=slot_t[:, GROUP_BOUNDS[g] : GROUP_BOUNDS[g + 1]],
                    )
                for g in range(NG):
                    nc.vector.max_index(
                        out=lo[:, g * 8 : (g + 1) * 8],
                        in_max=u[:, g * 8 : (g + 1) * 8],
                        in_values=slot_t[:, GROUP_BOUNDS[g] : GROUP_BOUNDS[g + 1]],
                    )
                rs = slice(t * P, (t + 1) * P)
                nc.sync.dma_start(l_ap[rs, :], lo[:])

            prev = None  # (t, slots tile) pending MAX8/FIND
            for t in range(n_tiles):
                qs = slice(t * P, (t + 1) * P)
                slot_t = spool.tile([P, SLOTS], f16, tag="slots")
                sbf = sbpool.tile([P, N], f16, tag="sbf")
                for h in range(2):
                    ps = ppool.tile([P, N // 2], f32, tag="ps")
                    for j in range(4):
                        cj = h * 4 + j
                        nc.tensor.matmul(
                            ps[:, j * 512 : (j + 1) * 512],
                            l_sb[:, qs], r_sb[cj][:, :],
                            start=True, stop=True,
                        )
                    # scalar engine stages the half into SBUF as fp16; the
                    # DVE max tree then runs in 2x (16-bit) mode
                    nc.scalar.copy(sbf[:, h * 2048 : (h + 1) * 2048], ps[:, :])
                # the pending MAX8/FIND of tile t-1 fills the DVE while the
                # scalar engine still stages tile t
                if prev is not None:
                    emit_maxfind(*prev)
                # max tree: fold (j, j + width/2) pairs; every level is one
                # flat contiguous instruction, so slot s = max over
                # columns s + SLOTS*k, k = 0..FMERGE-1
                m1 = m1pool.tile([P, 2048], f16, tag="m1")
                m2 = m2pool.tile([P, 1024], f16, tag="m2")
                m3 = m3pool.tile([P, 512], f16, tag="m3")
                for src, dst, w in (
                    (sbf, m1, 2048), (m1, m2, 1024), (m2, m3, 512), (m3, slot_t, 256),
                ):
                    nc.vector.tensor_max(
                        dst[:, 0:w], src[:, 0:w], src[:, w : 2 * w]
                    )
                prev = (t, slot_t)
            emit_maxfind(*prev)
    nc.compile()
    return nc


def _prep_core_inputs(X, core):
    """X: (B, N, C) fp32. Returns input map for one core."""
    b, h = divmod(core, N_CORES // B)
    Xb = X[b]
    xsq = np.sum(Xb * Xb, axis=1, dtype=np.float32)
    # 3-level fp16 split of (C0 - xsq) (exact to ~1e-6); the C0 bias
    # centers the top scores near 0 where the fp16 grid is finest
    t0 = C0 - xsq
    s1 = t0.astype(np.float16)
    r = t0 - s1.astype(np.float32)
    s2 = r.astype(np.float16)
    s3 = (r - s2.astype(np.float32)).astype(np.float16)
    rhs = np.empty((KM, N), np.float16)
    rhs[:C] = Xb.T.astype(np.float16)
    rhs[C] = s1
    rhs[C + 1] = s2
    rhs[C + 2] = s3
    lhs = np.empty((KM, QROWS), np.float16)
    lhs[:C] = (2.0 * Xb[h * QROWS : (h + 1) * QROWS]).T.astype(np.float16)
    lhs[C:] = 1.0
    return {"lhs": lhs, "rhs": rhs}


# base slot id of the group each of the UW output columns belongs to
_GROUP_BASE = np.asarray(GROUP_BOUNDS[:-1], dtype=np.int64)[np.arange(UW) // 8]


def _merge_core(L, Xb, xsq, h):
    """L: (QROWS, UW) u16 slot-local ids for one core. Returns
    (top18 (QROWS,18) int64, flagged row mask (QROWS,))."""
    R = L.shape[0]
    slots = L.astype(np.int64) + _GROUP_BASE[None, :]          # (R, UW)
    cand = (slots[:, :, None] + SLOTS * np.arange(FMERGE)[None, None, :]
            ).reshape(R, UW * FMERGE)                          # (R, 512)
    Q = 2.0 * Xb[h * QROWS : (h + 1) * QROWS]                  # (R, C)
    # exact scores: v[r,m] = <Q[r], X[cand]> - xsq[cand], chunked gather
    v = np.empty((R, UW * FMERGE), np.float32)
    CH = 512
    for r0 in range(0, R, CH):
        r1 = min(r0 + CH, R)
        Xg = Xb[cand[r0:r1]]                                   # (ch, 320, C)
        v[r0:r1] = np.matmul(Xg, Q[r0:r1, :, None])[..., 0]
    v -= xsq[cand]

    # stable jax-style ordering: by value desc, lowest column id on ties
    ord1 = np.argsort(cand, axis=1, kind="stable")
    cand1 = np.take_along_axis(cand, ord1, axis=1)
    v1 = np.take_along_axis(v, ord1, axis=1)
    ord2 = np.argsort(-v1, axis=1, kind="stable")
    top = np.take_along_axis(cand1, ord2, axis=1)[:, :K_EFF]
    v18 = np.take_along_axis(v1, ord2, axis=1)[:, K_EFF - 1]

    # flags
    slotmax = v.reshape(R, UW, FMERGE).max(axis=2)             # (R, UW)
    thr = (v18 - EPS)[:, None]
    cnt = (slotmax >= thr).reshape(R, NG, 8).sum(axis=2)       # (R, NG)
    flag_count = (cnt >= 8).any(axis=1)
    s_sorted = np.sort(slots.reshape(R, NG, 8), axis=2)
    sm_sorted = np.take_along_axis(
        slotmax.reshape(R, NG, 8), np.argsort(slots.reshape(R, NG, 8), axis=2), axis=2
    )
    dup = (np.diff(s_sorted, axis=2) == 0) & (sm_sorted[:, :, 1:] >= thr[:, :, None])
    flag_dup = dup.any(axis=(1, 2))
    return top, flag_count | flag_dup


_NC_CACHE = {}


def kernel(x: np.ndarray) -> np.ndarray:
    x = np.asarray(x)
    assert x.shape == (B, C, N, 1), x.shape
    X = np.ascontiguousarray(np.transpose(x[..., 0], (0, 2, 1)))  # (B, N, C)

    if N_TILES not in _NC_CACHE:
        _NC_CACHE[N_TILES] = _build_program(N_TILES)
    nc = _NC_CACHE[N_TILES]

    in_maps = [_prep_core_inputs(X, c) for c in range(N_CORES)]
    res = run_bass_kernel_spmd(nc, in_maps, core_ids=list(range(N_CORES)))

    xsqs = [np.sum(X[b] * X[b], axis=1, dtype=np.float32) for b in range(B)]
    nn_idx = np.empty((B, N, K_EFF), np.int64)
    bad_rows = [[] for _ in range(B)]
    for core in range(N_CORES):
        b, h = divmod(core, N_CORES // B)
        L = res.results[core]["l_out"]
        idx, bad = _merge_core(L, X[b], xsqs[b], h)
        nn_idx[b, h * QROWS : (h + 1) * QROWS] = idx
        if bad.any():
            bad_rows[b].extend((h * QROWS + np.nonzero(bad)[0]).tolist())

    # vectorized host repair of flagged rows (exact fp32 recompute)
    for b in range(B):
        if not bad_rows[b]:
            continue
        rows = np.asarray(sorted(bad_rows[b]))
        Xb = X[b]
        S = (2.0 * Xb[rows]) @ Xb.T
        S = (S - xsqs[b][None, :]).astype(np.float32)
        order = np.argsort(-S, axis=1, kind="stable")
        nn_idx[b, rows] = order[:, :K_EFF]

    nn_dil = nn_idx[:, :, ::DILATION]                       # (B, N, 9)
    center = np.broadcast_to(np.arange(N)[None, :, None], nn_dil.shape)
    out = np.stack((nn_dil, center), axis=0).astype(np.int32)
    return out
